# revision 9
# baseline (speedup 1.0000x reference)
"""Distributed trace-polynomial Bass kernel for trn2 (8 NeuronCores), v2.

Problem: x [65536,16,16] f32, coef [10,4].
  t_u(b) = trace(x_b^(u+2)), u=0..9
  out[b] = sum_uj coef[u,j] * t_u^(j+1) / 256^(u+j+1)

v2 math (per core, 8192 samples, bf16 tiles):
  Two chains meet in the middle; only 5 matmuls per 8-sample group
  (baseline needed 9):
    Z-chain (transposed powers, Z_a[(s,i),(c,j)] = x^a[j,i]):
      z2 = S[x] . z1, z3 = S[x] . z2
    W-chain (natural powers, W_b[(s,i),(c,j)] = x^b[i,j]):
      W2 = S[z1] . W1, W5 = S[z3] . W2, W8 = S[z3] . W5
  S[t] is an 8-sample block-diagonal stationary built from tile t.
  Traces: tr(x^(a+b))[b] = <Z_a, W_b>_F per sample; pairs (a,b) for
  k=2..11: (1,1) (1,2) (2,2) (3,2) (1,5) (2,5) (3,5) (1,8) (2,8) (3,8).
  Products+j-folds on DVE/Pool, i-fold on TE (N=64 f32 selector MMs),
  features on DVE (same endgame as baseline).

v2 layout: slabs stay [c][s][j] (the BIR verifier requires a single
free dim on the stationary AP, which forces contiguous 128-col group
slabs); expansion DMAs scatter 32B runs across the 16 DMA engines.
Host ships x and z pre-reordered to partition-major [ch][s][i][c][j]
so the compact loads are single contiguous 2KB-per-partition DMAs.

Sharding: pure data parallel, batch split 8 ways across cores.
"""

import numpy as np
from contextlib import ExitStack

import ml_dtypes

from concourse import bass, bacc, tile, mybir
from concourse.bass_utils import run_bass_kernel_spmd

B, N = 65536, 16
ROWS, COLS = 10, 4
M = 8           # cores
BS = B // M     # 8192 samples per core
CH = 16         # chunks per core
CS = BS // CH   # 512 samples per chunk
G = CS // 8     # 64 groups (of 8 samples) per chunk

BF16 = mybir.dt.bfloat16
F32 = mybir.dt.float32

# trace pairs: k = a + b, Z-side a in {1,2,3}, W-side b in {1,2,5,8}
PAIRS = [(1, 1), (1, 2), (2, 2), (3, 2), (1, 5), (2, 5), (3, 5),
         (1, 8), (2, 8), (3, 8)]

_cached = None


def _consts(coef: np.ndarray):
    # row layout of the 96-partition trace tile: r = 32*(u//4) + 8*(u%4) + s
    wmat = np.zeros((96, COLS), np.float32)
    self8 = np.zeros((96, 8), np.float32)
    for u in range(ROWS):
        base = 32 * (u // 4) + 8 * (u % 4)
        for s in range(8):
            wmat[base + s, :] = coef[u, :] * (256.0 ** (-u))
            self8[base + s, s] = 1.0
    # sel4[:, 32q:32q+32]: lhsT mapping partition (s,i) -> within-slice col 8q+s
    sel4 = np.zeros((128, 128), np.float32)
    for q in range(4):
        for s in range(8):
            for i in range(16):
                sel4[16 * s + i, 32 * q + 8 * q + s] = 1.0
    return sel4, self8, wmat


def build():
    nc = bacc.Bacc("TRN2", target_bir_lowering=False, debug=False, num_devices=M)

    # xr row (ch, s, i) = 128*ch + 16*s + i; col (c, j) = 16*c + j
    # xr[(ch,s,i), (c,j)] = x_{512ch+8c+s}[i, j]
    # zr[(ch,s,k), (c,i)] = x_{512ch+8c+s}[i, k]
    x_d = nc.dram_tensor("x", [CH * 128, 1024], BF16, kind="ExternalInput").ap()
    z_d = nc.dram_tensor("z", [CH * 128, 1024], BF16, kind="ExternalInput").ap()
    sel4_d = nc.dram_tensor("sel4", [128, 128], F32, kind="ExternalInput").ap()
    self8_d = nc.dram_tensor("self8", [96, 8], F32, kind="ExternalInput").ap()
    wmat_d = nc.dram_tensor("wmat", [96, COLS], F32, kind="ExternalInput").ap()
    out_d = nc.dram_tensor("out", [BS], F32, kind="ExternalOutput").ap()

    with tile.TileContext(nc) as tc:
        with ExitStack() as ctx:
            consts = ctx.enter_context(tc.tile_pool(name="consts", bufs=1))
            xexp_p = ctx.enter_context(tc.tile_pool(name="xexp", bufs=1))
            data = ctx.enter_context(tc.tile_pool(name="data", bufs=2))
            pows = ctx.enter_context(tc.tile_pool(name="pows", bufs=2))
            prod = ctx.enter_context(tc.tile_pool(name="prod", bufs=3))
            pfold = ctx.enter_context(tc.tile_pool(name="pfold", bufs=2))
            psum_c = ctx.enter_context(tc.tile_pool(name="psumc", bufs=2, space="PSUM"))
            psum_t = ctx.enter_context(tc.tile_pool(name="psumt", bufs=2, space="PSUM"))
            trc = ctx.enter_context(tc.tile_pool(name="trace", bufs=1))
            fin = ctx.enter_context(tc.tile_pool(name="fin", bufs=1))

            sel4_t = consts.tile([128, 128], F32)
            nc.sync.dma_start(sel4_t[:], sel4_d[:])
            self8_t = consts.tile([96, 8], F32)
            nc.sync.dma_start(self8_t[:], self8_d[:])
            wmat_t = consts.tile([96, COLS], F32)
            nc.sync.dma_start(wmat_t[:], wmat_d[:])

            # trace accumulator [96, CH*G] f32
            T_t = trc.tile([96, CH * G], F32)

            # block-diag stationary buffers, [s-block][c][j] column order,
            # manually double buffered; off-block zeros written once persist.
            def expbuf(name):
                ts = [
                    xexp_p.tile([128, 8 * 1024], BF16, tag=f"{name}{i}",
                                name=f"{name}{i}")
                    for i in range(2)
                ]
                return ts

            xe_ts = expbuf("xe")
            ze_ts = expbuf("ze")
            z3e_ts = expbuf("z3e")
            eng_ms = [nc.gpsimd, nc.vector, nc.gpsimd, nc.vector, nc.gpsimd,
                      nc.vector]
            for t, e in zip(xe_ts + ze_ts + z3e_ts, eng_ms):
                e.memset(t[:], 0.0)

            # lhsT view for group c: contiguous 128-col block-diag slab
            def exp_view(t):
                return t.rearrange("p (c w) -> p c w", w=128)

            for ch in range(CH):
                xe, ze, z3e = xe_ts[ch % 2], ze_ts[ch % 2], z3e_ts[ch % 2]
                xe_v, ze_v, z3e_v = exp_view(xe), exp_view(ze), exp_view(z3e)  # [p, c, 128]
                xrows = x_d[128 * ch: 128 * (ch + 1), :]
                zrows = z_d[128 * ch: 128 * (ch + 1), :]

                # compact moving tiles (one contiguous DMA each)
                xc_t = data.tile([128, 1024], BF16, tag="xc")
                nc.sync.dma_start(xc_t[:], xrows)
                zc_t = data.tile([128, 1024], BF16, tag="zc")
                nc.sync.dma_start(zc_t[:], zrows)

                # block-diag slices: slice s scatters into slot s of each
                # group slab (32B runs; packets spread over 16 DMA engines)
                xe_r, ze_r = exp_view(xe), exp_view(ze)
                qs = [nc.sync, nc.gpsimd, nc.scalar]
                for s in range(8):
                    qs[s % 3].dma_start(
                        xe_r[16 * s: 16 * s + 16, :, 16 * s: 16 * s + 16],
                        xrows[16 * s: 16 * s + 16, :].rearrange(
                            "p (c j) -> p c j", j=16),
                    )
                    qs[(s + 1) % 3].dma_start(
                        ze_r[16 * s: 16 * s + 16, :, 16 * s: 16 * s + 16],
                        zrows[16 * s: 16 * s + 16, :].rearrange(
                            "p (c j) -> p c j", j=16),
                    )

                tiles = {}          # power tiles in SBUF, bf16
                tiles[("Z", 1)] = zc_t
                tiles[("W", 1)] = xc_t

                Pf_t = pfold.tile([128, 640], F32, tag="pf")
                ps_tr = psum_t.tile([96, G], F32, tag="pstr")
                prod_state = {"n": 0}

                def chain(side, p, stat_v, rhs_key, copy_split):
                    ps = psum_c.tile([128, 1024], F32, tag="psz")
                    rhs = tiles[rhs_key]
                    for c in range(G):
                        nc.tensor.matmul(
                            ps[:, 16 * c: 16 * c + 16],
                            stat_v[:, c],
                            rhs[:, 16 * c: 16 * c + 16],
                            start=True,
                            stop=True,
                        )
                    t = pows.tile([128, 1024], BF16, tag=f"{side}{p}")
                    if copy_split:
                        HW = 512
                        nc.scalar.copy(t[:, 0:HW], ps[:, 0:HW])
                        copy_split(t[:, HW:], ps[:, HW:])
                    else:
                        nc.scalar.copy(t[:], ps[:])
                    tiles[(side, p)] = t

                def emit_products(ready):
                    # products for pairs whose operands are both ready
                    for u, (a, b) in enumerate(PAIRS):
                        if (("Z", a) in tiles and ("W", b) in tiles
                                and u not in prod_state):
                            prod_state[u] = True
                            prod_state["n"] += 1
                            P_t = prod.tile([128, 1024], BF16, tag="pair")
                            nc.gpsimd.tensor_tensor(
                                P_t[:], tiles[("Z", a)][:], tiles[("W", b)][:],
                                mybir.AluOpType.mult,
                            )
                            nc.vector.tensor_reduce(
                                Pf_t[:, 64 * u: 64 * u + 64],
                                P_t.rearrange("p (c j) -> p c j", j=16),
                                mybir.AxisListType.X,
                                mybir.AluOpType.add,
                            )
                            # i-fold into trace strips: row 8q+s of strip
                            strip, q = u // 4, u % 4
                            qlast = 3 if strip < 2 else 1
                            nc.tensor.matmul(
                                ps_tr[32 * strip: 32 * strip + 32, :],
                                sel4_t[:, 32 * q: 32 * q + 32],
                                Pf_t[:, 64 * u: 64 * u + 64],
                                start=(q == 0),
                                stop=(q == qlast),
                                tile_position=(0, 32 * strip),
                            )

                chain("Z", 2, xe_v, ("Z", 1), nc.vector.tensor_copy)
                emit_products(tiles)
                chain("W", 2, ze_v, ("W", 1), nc.vector.tensor_copy)
                emit_products(tiles)
                chain("Z", 3, xe_v, ("Z", 2), nc.vector.tensor_copy)
                emit_products(tiles)

                # expand z3 into block-diag (SBUF->SBUF, contiguous 2KB runs)
                z3_t = tiles[("Z", 3)]
                z3e_r = exp_view(z3e)
                for s in range(8):
                    qs[s % 3].dma_start(
                        z3e_r[16 * s: 16 * s + 16, :, 16 * s: 16 * s + 16],
                        z3_t[16 * s: 16 * s + 16, :].rearrange(
                            "p (c j) -> p c j", j=16),
                    )

                chain("W", 5, z3e_v, ("W", 2), nc.vector.tensor_copy)
                emit_products(tiles)
                chain("W", 8, z3e_v, ("W", 5), None)
                emit_products(tiles)

                # stash this chunk's traces
                nc.scalar.copy(T_t[:, G * ch: G * (ch + 1)], ps_tr[:])

            # features: S = T/256, G_acc = sum_j W[:,j] * S^(j+1)
            S_t = fin.tile([96, CH * G], F32, tag="S")
            nc.vector.tensor_scalar_mul(S_t[:], T_t[:], 1.0 / 256.0)
            S2_t = fin.tile([96, CH * G], F32, tag="S2")
            nc.vector.tensor_tensor(S2_t[:], S_t[:], S_t[:], mybir.AluOpType.mult)
            S3_t = fin.tile([96, CH * G], F32, tag="S3")
            nc.gpsimd.tensor_tensor(S3_t[:], S2_t[:], S_t[:], mybir.AluOpType.mult)
            S4_t = fin.tile([96, CH * G], F32, tag="S4")
            nc.vector.tensor_tensor(S4_t[:], S2_t[:], S2_t[:], mybir.AluOpType.mult)

            G1_t = fin.tile([96, CH * G], F32, tag="G1")
            nc.vector.tensor_scalar(
                G1_t[:], S_t[:], wmat_t[:, 0:1], None, mybir.AluOpType.mult
            )
            G2_t = fin.tile([96, CH * G], F32, tag="G2")
            nc.vector.scalar_tensor_tensor(
                G2_t[:], S2_t[:], wmat_t[:, 1:2], G1_t[:],
                mybir.AluOpType.mult, mybir.AluOpType.add,
            )
            G3_t = fin.tile([96, CH * G], F32, tag="G3")
            nc.vector.scalar_tensor_tensor(
                G3_t[:], S3_t[:], wmat_t[:, 2:3], G2_t[:],
                mybir.AluOpType.mult, mybir.AluOpType.add,
            )
            G4_t = fin.tile([96, CH * G], F32, tag="G4")
            nc.vector.scalar_tensor_tensor(
                G4_t[:], S4_t[:], wmat_t[:, 3:4], G3_t[:],
                mybir.AluOpType.mult, mybir.AluOpType.add,
            )

            # fold the 96 rows into 8 sample rows: out[s, (ch,c)]
            ps_out = psum_c.tile([8, CH * G], F32, tag="psz", name="ps_out")
            for h in range(0, CH * G, 512):
                w = min(512, CH * G - h)
                nc.tensor.matmul(
                    ps_out[:, h: h + w],
                    self8_t[:],
                    G4_t[:, h: h + w],
                    start=True,
                    stop=True,
                )
            out_sb = fin.tile([8, CH * G], F32, tag="outsb")
            nc.vector.tensor_copy(out_sb[:], ps_out[:])
            # out[b], b = 512*ch + 8*c + s; cols are (ch,c)
            nc.sync.dma_start(
                out_d.rearrange("(ch c s) -> s (ch c)", ch=CH, c=G, s=8),
                out_sb[:],
            )

    nc.compile()
    return nc


def _prep_inputs(x: np.ndarray, coef: np.ndarray):
    x = np.ascontiguousarray(x, dtype=np.float32).reshape(B, N, N)
    xb = x.astype(ml_dtypes.bfloat16)
    # [core, ch, c, s, i, j]
    x6 = xb.reshape(M, CH, G, 8, N, N)
    xr = np.ascontiguousarray(x6.transpose(0, 1, 3, 4, 2, 5)).reshape(
        M, CH * 128, 1024
    )
    # zr[(ch,s,k),(c,i)] = x[ch,c,s,i,k]
    zr = np.ascontiguousarray(x6.transpose(0, 1, 3, 5, 2, 4)).reshape(
        M, CH * 128, 1024
    )
    sel4, self8, wmat = _consts(np.asarray(coef, dtype=np.float32))
    return [
        {"x": xr[i], "z": zr[i], "sel4": sel4, "self8": self8, "wmat": wmat}
        for i in range(M)
    ]


def kernel(x: np.ndarray, coef: np.ndarray) -> np.ndarray:
    global _cached
    if _cached is None:
        _cached = build()
    in_maps = _prep_inputs(x, coef)
    res = run_bass_kernel_spmd(_cached, in_maps, core_ids=list(range(M)))
    out = np.concatenate(
        [np.asarray(res.results[i]["out"]).reshape(BS) for i in range(M)]
    )
    return out.astype(np.float32)


if __name__ == "__main__":
    rng = np.random.default_rng(0)
    x = (rng.standard_normal((B, N, N)) * 0.5).astype(np.float32)
    coef = (rng.standard_normal((ROWS, COLS)) * np.sqrt(0.5)).astype(np.float32)
    got = kernel(x, coef)
    print(got[:8])


# revision 10
# speedup vs baseline: 1.1082x; 1.1082x over previous
"""Distributed trace-polynomial Bass kernel for trn2 (8 NeuronCores), v2.

Problem: x [65536,16,16] f32, coef [10,4].
  t_u(b) = trace(x_b^(u+2)), u=0..9
  out[b] = sum_uj coef[u,j] * t_u^(j+1) / 256^(u+j+1)

v2 math (per core, 8192 samples, bf16 tiles):
  Two chains meet in the middle; only 5 matmuls per 8-sample group
  (baseline needed 9):
    Z-chain (transposed powers, Z_a[(s,i),(c,j)] = x^a[j,i]):
      z2 = S[x] . z1, z3 = S[x] . z2
    W-chain (natural powers, W_b[(s,i),(c,j)] = x^b[i,j]):
      W2 = S[z1] . W1, W5 = S[z3] . W2, W8 = S[z3] . W5
  S[t] is an 8-sample block-diagonal stationary built from tile t.
  Traces: tr(x^(a+b))[b] = <Z_a, W_b>_F per sample; pairs (a,b) for
  k=2..11: (1,1) (1,2) (2,2) (3,2) (1,5) (2,5) (3,5) (1,8) (2,8) (3,8).
  Products+j-folds on DVE/Pool, i-fold on TE (N=64 f32 selector MMs),
  features on DVE (same endgame as baseline).

v2 layout: slabs stay [c][s][j] (the BIR verifier requires a single
free dim on the stationary AP, which forces contiguous 128-col group
slabs); expansion DMAs scatter 32B runs across the 16 DMA engines.
Host ships x and z pre-reordered to partition-major [ch][s][i][c][j]
so the compact loads are single contiguous 2KB-per-partition DMAs.

Sharding: pure data parallel, batch split 8 ways across cores.
"""

import numpy as np
from contextlib import ExitStack

import ml_dtypes

from concourse import bass, bacc, tile, mybir
from concourse.bass_utils import run_bass_kernel_spmd

B, N = 65536, 16
ROWS, COLS = 10, 4
M = 8           # cores
BS = B // M     # 8192 samples per core
CH = 16         # chunks per core
CS = BS // CH   # 512 samples per chunk
G = CS // 8     # 64 groups (of 8 samples) per chunk

BF16 = mybir.dt.bfloat16
F32 = mybir.dt.float32

# trace pairs: k = a + b, Z-side a in {1,2,3}, W-side b in {1,2,5,8}
PAIRS = [(1, 1), (1, 2), (2, 2), (3, 2), (1, 5), (2, 5), (3, 5),
         (1, 8), (2, 8), (3, 8)]

_cached = None


def _consts(coef: np.ndarray):
    # row layout of the 96-partition trace tile: r = 32*(u//4) + 8*(u%4) + s
    wmat = np.zeros((96, COLS), np.float32)
    self8 = np.zeros((96, 8), np.float32)
    for u in range(ROWS):
        base = 32 * (u // 4) + 8 * (u % 4)
        for s in range(8):
            wmat[base + s, :] = coef[u, :] * (256.0 ** (-u))
            self8[base + s, s] = 1.0
    # sel4[:, 32q:32q+32]: lhsT mapping partition (s,i) -> within-slice col 8q+s
    sel4 = np.zeros((128, 128), np.float32)
    for q in range(4):
        for s in range(8):
            for i in range(16):
                sel4[16 * s + i, 32 * q + 8 * q + s] = 1.0
    return sel4.astype(ml_dtypes.bfloat16), self8, wmat


def build():
    nc = bacc.Bacc("TRN2", target_bir_lowering=False, debug=False, num_devices=M)

    # xr row (ch, s, i) = 128*ch + 16*s + i; col (c, j) = 16*c + j
    # xr[(ch,s,i), (c,j)] = x_{512ch+8c+s}[i, j]
    # zr[(ch,s,k), (c,i)] = x_{512ch+8c+s}[i, k]
    x_d = nc.dram_tensor("x", [CH * 128, 1024], BF16, kind="ExternalInput").ap()
    z_d = nc.dram_tensor("z", [CH * 128, 1024], BF16, kind="ExternalInput").ap()
    sel4_d = nc.dram_tensor("sel4", [128, 128], BF16, kind="ExternalInput").ap()
    self8_d = nc.dram_tensor("self8", [96, 8], F32, kind="ExternalInput").ap()
    wmat_d = nc.dram_tensor("wmat", [96, COLS], F32, kind="ExternalInput").ap()
    out_d = nc.dram_tensor("out", [BS], F32, kind="ExternalOutput").ap()

    with tile.TileContext(nc) as tc:
        with ExitStack() as ctx:
            consts = ctx.enter_context(tc.tile_pool(name="consts", bufs=1))
            xexp_p = ctx.enter_context(tc.tile_pool(name="xexp", bufs=1))
            data = ctx.enter_context(tc.tile_pool(name="data", bufs=2))
            pows = ctx.enter_context(tc.tile_pool(name="pows", bufs=2))
            prod = ctx.enter_context(tc.tile_pool(name="prod", bufs=3))
            psum_c = ctx.enter_context(tc.tile_pool(name="psumc", bufs=2, space="PSUM"))
            psum_t = ctx.enter_context(tc.tile_pool(name="psumt", bufs=2, space="PSUM"))
            trc = ctx.enter_context(tc.tile_pool(name="trace", bufs=1))
            fin = ctx.enter_context(tc.tile_pool(name="fin", bufs=1))

            sel4_t = consts.tile([128, 128], BF16)
            nc.sync.dma_start(sel4_t[:], sel4_d[:])
            self8_t = consts.tile([96, 8], F32)
            nc.sync.dma_start(self8_t[:], self8_d[:])
            wmat_t = consts.tile([96, COLS], F32)
            nc.sync.dma_start(wmat_t[:], wmat_d[:])

            # trace accumulator [96, CH*G] f32
            T_t = trc.tile([96, CH * G], F32)

            # block-diag stationary buffers, [s-block][c][j] column order,
            # manually double buffered; off-block zeros written once persist.
            def expbuf(name):
                ts = [
                    xexp_p.tile([128, 8 * 1024], BF16, tag=f"{name}{i}",
                                name=f"{name}{i}")
                    for i in range(2)
                ]
                return ts

            xe_ts = expbuf("xe")
            ze_ts = expbuf("ze")
            z3e_ts = expbuf("z3e")
            eng_ms = [nc.gpsimd, nc.vector, nc.gpsimd, nc.vector, nc.gpsimd,
                      nc.vector]
            for t, e in zip(xe_ts + ze_ts + z3e_ts, eng_ms):
                e.memset(t[:], 0.0)

            # lhsT view for group c: contiguous 128-col block-diag slab
            def exp_view(t):
                return t.rearrange("p (c w) -> p c w", w=128)

            for ch in range(CH):
                xe, ze, z3e = xe_ts[ch % 2], ze_ts[ch % 2], z3e_ts[ch % 2]
                xe_v, ze_v, z3e_v = exp_view(xe), exp_view(ze), exp_view(z3e)  # [p, c, 128]
                xrows = x_d[128 * ch: 128 * (ch + 1), :]
                zrows = z_d[128 * ch: 128 * (ch + 1), :]

                # compact moving tiles (one contiguous DMA each)
                xc_t = data.tile([128, 1024], BF16, tag="xc")
                nc.sync.dma_start(xc_t[:], xrows)
                zc_t = data.tile([128, 1024], BF16, tag="zc")
                nc.sync.dma_start(zc_t[:], zrows)

                # block-diag slices: slice s scatters into slot s of each
                # group slab (32B runs; packets spread over 16 DMA engines)
                xe_r, ze_r = exp_view(xe), exp_view(ze)
                qs = [nc.sync, nc.gpsimd, nc.scalar]
                for s in range(8):
                    qs[s % 3].dma_start(
                        xe_r[16 * s: 16 * s + 16, :, 16 * s: 16 * s + 16],
                        xrows[16 * s: 16 * s + 16, :].rearrange(
                            "p (c j) -> p c j", j=16),
                    )
                    qs[(s + 1) % 3].dma_start(
                        ze_r[16 * s: 16 * s + 16, :, 16 * s: 16 * s + 16],
                        zrows[16 * s: 16 * s + 16, :].rearrange(
                            "p (c j) -> p c j", j=16),
                    )

                tiles = {}          # power tiles in SBUF, bf16
                tiles[("Z", 1)] = zc_t
                tiles[("W", 1)] = xc_t

                ps_tr = psum_t.tile([96, 16 * G], F32, tag="pstr")
                prod_state = {"n": 0}

                def chain(side, p, stat_v, rhs_key, copy_split):
                    ps = psum_c.tile([128, 1024], F32, tag="psz")
                    rhs = tiles[rhs_key]
                    for c in range(G):
                        nc.tensor.matmul(
                            ps[:, 16 * c: 16 * c + 16],
                            stat_v[:, c],
                            rhs[:, 16 * c: 16 * c + 16],
                            start=True,
                            stop=True,
                        )
                    t = pows.tile([128, 1024], BF16, tag=f"{side}{p}")
                    if copy_split:
                        HW = 512
                        nc.scalar.copy(t[:, 0:HW], ps[:, 0:HW])
                        copy_split(t[:, HW:], ps[:, HW:])
                    else:
                        nc.scalar.copy(t[:], ps[:])
                    tiles[(side, p)] = t

                def emit_products(ready):
                    # products for pairs whose operands are both ready
                    for u, (a, b) in enumerate(PAIRS):
                        if (("Z", a) in tiles and ("W", b) in tiles
                                and u not in prod_state):
                            prod_state[u] = True
                            prod_state["n"] += 1
                            P_t = prod.tile([128, 1024], BF16, tag="pair")
                            nc.vector.tensor_tensor(
                                P_t[:], tiles[("Z", a)][:], tiles[("W", b)][:],
                                mybir.AluOpType.mult,
                            )
                            # i-fold into trace strips: row 8q+s of strip,
                            # (c,j) cols kept; j folded once per chunk
                            strip, q = u // 4, u % 4
                            qlast = 3 if strip < 2 else 1
                            for h in range(0, 16 * G, 512):
                                nc.tensor.matmul(
                                    ps_tr[32 * strip: 32 * strip + 32,
                                          h: h + 512],
                                    sel4_t[:, 32 * q: 32 * q + 32],
                                    P_t[:, h: h + 512],
                                    start=(q == 0),
                                    stop=(q == qlast),
                                    tile_position=(0, 32 * strip),
                                )

                chain("Z", 2, xe_v, ("Z", 1), nc.vector.tensor_copy)
                emit_products(tiles)
                chain("W", 2, ze_v, ("W", 1), nc.vector.tensor_copy)
                emit_products(tiles)
                chain("Z", 3, xe_v, ("Z", 2), nc.vector.tensor_copy)
                emit_products(tiles)

                # expand z3 into block-diag (SBUF->SBUF, contiguous 2KB runs)
                z3_t = tiles[("Z", 3)]
                z3e_r = exp_view(z3e)
                for s in range(8):
                    qs[s % 3].dma_start(
                        z3e_r[16 * s: 16 * s + 16, :, 16 * s: 16 * s + 16],
                        z3_t[16 * s: 16 * s + 16, :].rearrange(
                            "p (c j) -> p c j", j=16),
                    )

                chain("W", 5, z3e_v, ("W", 2), nc.vector.tensor_copy)
                emit_products(tiles)
                chain("W", 8, z3e_v, ("W", 5), None)
                emit_products(tiles)

                # j-fold 16-col segments: [96, (c,j)] -> [96, c]
                nc.vector.tensor_reduce(
                    T_t[:, G * ch: G * (ch + 1)],
                    ps_tr.rearrange("p (c j) -> p c j", j=16),
                    mybir.AxisListType.X,
                    mybir.AluOpType.add,
                )

            # features: S = T/256, G_acc = sum_j W[:,j] * S^(j+1)
            S_t = fin.tile([96, CH * G], F32, tag="S")
            nc.vector.tensor_scalar_mul(S_t[:], T_t[:], 1.0 / 256.0)
            S2_t = fin.tile([96, CH * G], F32, tag="S2")
            nc.vector.tensor_tensor(S2_t[:], S_t[:], S_t[:], mybir.AluOpType.mult)
            S3_t = fin.tile([96, CH * G], F32, tag="S3")
            nc.gpsimd.tensor_tensor(S3_t[:], S2_t[:], S_t[:], mybir.AluOpType.mult)
            S4_t = fin.tile([96, CH * G], F32, tag="S4")
            nc.vector.tensor_tensor(S4_t[:], S2_t[:], S2_t[:], mybir.AluOpType.mult)

            G1_t = fin.tile([96, CH * G], F32, tag="G1")
            nc.vector.tensor_scalar(
                G1_t[:], S_t[:], wmat_t[:, 0:1], None, mybir.AluOpType.mult
            )
            G2_t = fin.tile([96, CH * G], F32, tag="G2")
            nc.vector.scalar_tensor_tensor(
                G2_t[:], S2_t[:], wmat_t[:, 1:2], G1_t[:],
                mybir.AluOpType.mult, mybir.AluOpType.add,
            )
            G3_t = fin.tile([96, CH * G], F32, tag="G3")
            nc.vector.scalar_tensor_tensor(
                G3_t[:], S3_t[:], wmat_t[:, 2:3], G2_t[:],
                mybir.AluOpType.mult, mybir.AluOpType.add,
            )
            G4_t = fin.tile([96, CH * G], F32, tag="G4")
            nc.vector.scalar_tensor_tensor(
                G4_t[:], S4_t[:], wmat_t[:, 3:4], G3_t[:],
                mybir.AluOpType.mult, mybir.AluOpType.add,
            )

            # fold the 96 rows into 8 sample rows: out[s, (ch,c)]
            ps_out = psum_c.tile([8, CH * G], F32, tag="psz", name="ps_out")
            for h in range(0, CH * G, 512):
                w = min(512, CH * G - h)
                nc.tensor.matmul(
                    ps_out[:, h: h + w],
                    self8_t[:],
                    G4_t[:, h: h + w],
                    start=True,
                    stop=True,
                )
            out_sb = fin.tile([8, CH * G], F32, tag="outsb")
            nc.vector.tensor_copy(out_sb[:], ps_out[:])
            # out[b], b = 512*ch + 8*c + s; cols are (ch,c)
            nc.sync.dma_start(
                out_d.rearrange("(ch c s) -> s (ch c)", ch=CH, c=G, s=8),
                out_sb[:],
            )

    nc.compile()
    return nc


def _prep_inputs(x: np.ndarray, coef: np.ndarray):
    x = np.ascontiguousarray(x, dtype=np.float32).reshape(B, N, N)
    xb = x.astype(ml_dtypes.bfloat16)
    # [core, ch, c, s, i, j]
    x6 = xb.reshape(M, CH, G, 8, N, N)
    xr = np.ascontiguousarray(x6.transpose(0, 1, 3, 4, 2, 5)).reshape(
        M, CH * 128, 1024
    )
    # zr[(ch,s,k),(c,i)] = x[ch,c,s,i,k]
    zr = np.ascontiguousarray(x6.transpose(0, 1, 3, 5, 2, 4)).reshape(
        M, CH * 128, 1024
    )
    sel4, self8, wmat = _consts(np.asarray(coef, dtype=np.float32))
    return [
        {"x": xr[i], "z": zr[i], "sel4": sel4, "self8": self8, "wmat": wmat}
        for i in range(M)
    ]


def kernel(x: np.ndarray, coef: np.ndarray) -> np.ndarray:
    global _cached
    if _cached is None:
        _cached = build()
    in_maps = _prep_inputs(x, coef)
    res = run_bass_kernel_spmd(_cached, in_maps, core_ids=list(range(M)))
    out = np.concatenate(
        [np.asarray(res.results[i]["out"]).reshape(BS) for i in range(M)]
    )
    return out.astype(np.float32)


if __name__ == "__main__":
    rng = np.random.default_rng(0)
    x = (rng.standard_normal((B, N, N)) * 0.5).astype(np.float32)
    coef = (rng.standard_normal((ROWS, COLS)) * np.sqrt(0.5)).astype(np.float32)
    got = kernel(x, coef)
    print(got[:8])


# revision 11
# speedup vs baseline: 1.3085x; 1.1807x over previous
"""Distributed trace-polynomial Bass kernel for trn2 (8 NeuronCores), v4.

Problem: x [65536,16,16] f32, coef [10,4].
  t_u(b) = trace(x_b^(u+2)), u=0..9
  out[b] = sum_uj coef[u,j] * t_u^(j+1) / 256^(u+j+1)

Math (per core, 8192 samples, bf16 tiles): two chains meet in the
middle, 5 matmuls per 8-sample group (baseline needed 9):
  Z-chain (transposed powers, Z_a[(s,i),(c,j)] = x^a[j,i]):
      z2 = S[x].z1,  z3 = S[x].z2,  z4 = S[x].z3
  W-chain (natural powers, W_b[(s,i),(c,j)] = x^b[i,j]):
      W4 = S[z3].x,  W7 = S[z3].W4
S[t] is an 8-sample block-diagonal stationary slab built from tile t.
Only TWO slab types: S[x] (filled from DRAM) and S[z3] (filled on-chip
from the z3 tile); slabs are triple-buffered so the 32B-segment
scatter DMAs have two chunk-periods to land.
Traces: tr(x^(a+b))[b] = <Z_a, W_b>_F per sample, pairs
  k=2..11: (1,1)(2,1)(3,1)(1,4)(2,4)(3,4)(1,7)(2,7)(3,7)(4,7).
Products on DVE (2 early ones on Pool), i-fold on TE via sel4
selector matmuls accumulating [96,(c,j)] strips in PSUM, one j-fold
tensor_reduce per chunk, feature endgame on DVE.

Host ships x and z=x^T pre-reordered to partition-major
[ch][s][i][c][j] so compact loads are single contiguous DMAs.
Sharding: pure data parallel, batch split 8 ways across cores.
"""

import numpy as np
from contextlib import ExitStack

import ml_dtypes

from concourse import bass, bacc, tile, mybir
from concourse.bass_utils import run_bass_kernel_spmd

B, N = 65536, 16
ROWS, COLS = 10, 4
M = 8           # cores
BS = B // M     # 8192 samples per core
CH = 16         # chunks per core
CS = BS // CH   # 512 samples per chunk
G = CS // 8     # 64 groups (of 8 samples) per chunk

BF16 = mybir.dt.bfloat16
F32 = mybir.dt.float32

# trace pairs: k = a + b, Z-side a in {1,2,3,4}, W-side b in {1,4,7}
PAIRS = [(1, 1), (2, 1), (3, 1), (1, 4), (2, 4), (3, 4),
         (1, 7), (2, 7), (3, 7), (4, 7)]

_cached = None


def _consts(coef: np.ndarray):
    # row layout of the 96-partition trace tile: r = 32*(u//4) + 8*(u%4) + s
    wmat = np.zeros((96, COLS), np.float32)
    self8 = np.zeros((96, 8), np.float32)
    for u in range(ROWS):
        base = 32 * (u // 4) + 8 * (u % 4)
        for s in range(8):
            wmat[base + s, :] = coef[u, :] * (256.0 ** (-u))
            self8[base + s, s] = 1.0
    # sel4[:, 32q:32q+32]: lhsT mapping partition (s,i) -> within-slice col 8q+s
    sel4 = np.zeros((128, 128), np.float32)
    for q in range(4):
        for s in range(8):
            for i in range(16):
                sel4[16 * s + i, 32 * q + 8 * q + s] = 1.0
    return sel4.astype(ml_dtypes.bfloat16), self8, wmat


def build():
    nc = bacc.Bacc("TRN2", target_bir_lowering=False, debug=False, num_devices=M)

    # xr row (ch, s, i) = 128*ch + 16*s + i; col (c, j) = 16*c + j
    # xr[(ch,s,i), (c,j)] = x_{512ch+8c+s}[i, j]
    # zr[(ch,s,k), (c,i)] = x_{512ch+8c+s}[i, k]
    x_d = nc.dram_tensor("x", [CH * 128, 1024], BF16, kind="ExternalInput").ap()
    z_d = nc.dram_tensor("z", [CH * 128, 1024], BF16, kind="ExternalInput").ap()
    sel4_d = nc.dram_tensor("sel4", [128, 128], BF16, kind="ExternalInput").ap()
    self8_d = nc.dram_tensor("self8", [96, 8], F32, kind="ExternalInput").ap()
    wmat_d = nc.dram_tensor("wmat", [96, COLS], F32, kind="ExternalInput").ap()
    out_d = nc.dram_tensor("out", [BS], F32, kind="ExternalOutput").ap()

    NB = 3  # slab buffers

    with tile.TileContext(nc) as tc:
        with ExitStack() as ctx:
            consts = ctx.enter_context(tc.tile_pool(name="consts", bufs=1))
            xexp_p = ctx.enter_context(tc.tile_pool(name="xexp", bufs=1))
            data = ctx.enter_context(tc.tile_pool(name="data", bufs=3))
            pows = ctx.enter_context(tc.tile_pool(name="pows", bufs=2))
            prod = ctx.enter_context(tc.tile_pool(name="prod", bufs=4))
            psum_c = ctx.enter_context(tc.tile_pool(name="psumc", bufs=2, space="PSUM"))
            psum_t = ctx.enter_context(tc.tile_pool(name="psumt", bufs=2, space="PSUM"))
            trc = ctx.enter_context(tc.tile_pool(name="trace", bufs=1))
            fin = ctx.enter_context(tc.tile_pool(name="fin", bufs=1))

            sel4_t = consts.tile([128, 128], BF16)
            nc.sync.dma_start(sel4_t[:], sel4_d[:])
            self8_t = consts.tile([96, 8], F32)
            nc.sync.dma_start(self8_t[:], self8_d[:])
            wmat_t = consts.tile([96, COLS], F32)
            nc.sync.dma_start(wmat_t[:], wmat_d[:])

            # trace accumulator [96, CH*G] f32
            T_t = trc.tile([96, CH * G], F32)

            # block-diag stationary slabs ([c][s][j] cols; BIR wants a single
            # free dim on the stationary, so group slabs are contiguous
            # 128-col blocks); off-block zeros are written once and persist.
            xe_ts = [xexp_p.tile([128, G * 128], BF16, tag=f"xe{i}", name=f"xe{i}")
                     for i in range(NB)]
            z3e_ts = [xexp_p.tile([128, G * 128], BF16, tag=f"z3e{i}",
                                  name=f"z3e{i}") for i in range(NB)]
            for i, t in enumerate(xe_ts + z3e_ts):
                [nc.gpsimd, nc.vector][i % 2].memset(t[:], 0.0)

            def exp_view(t):
                return t.rearrange("p (c w) -> p c w", w=128)

            qs = [nc.sync, nc.gpsimd, nc.scalar]

            for ch in range(CH):
                xe, z3e = xe_ts[ch % NB], z3e_ts[ch % NB]
                xe_v, z3e_v = exp_view(xe), exp_view(z3e)
                xrows = x_d[128 * ch: 128 * (ch + 1), :]
                zrows = z_d[128 * ch: 128 * (ch + 1), :]

                # compact moving tiles (one contiguous DMA each)
                xc_t = data.tile([128, 1024], BF16, tag="xc")
                nc.sync.dma_start(xc_t[:], xrows)
                zc_t = data.tile([128, 1024], BF16, tag="zc")
                nc.gpsimd.dma_start(zc_t[:], zrows)

                # S[x] slab: slice s scatters into slot s of each group slab
                for s in range(8):
                    qs[s % 3].dma_start(
                        xe_v[16 * s: 16 * s + 16, :, 16 * s: 16 * s + 16],
                        xrows[16 * s: 16 * s + 16, :].rearrange(
                            "p (c j) -> p c j", j=16),
                    )

                tiles = {("Z", 1): zc_t, ("W", 1): xc_t}
                ps_tr = psum_t.tile([96, 16 * G], F32, tag="pstr")
                prod_state = {"n": 0}

                def chain(side, p, stat_v, rhs_key, copy_split):
                    ps = psum_c.tile([128, 1024], F32, tag="psz")
                    rhs = tiles[rhs_key]
                    for c in range(G):
                        nc.tensor.matmul(
                            ps[:, 16 * c: 16 * c + 16],
                            stat_v[:, c],
                            rhs[:, 16 * c: 16 * c + 16],
                            start=True,
                            stop=True,
                        )
                    t = pows.tile([128, 1024], BF16, tag=f"{side}{p}")
                    if copy_split:
                        nc.scalar.copy(t[:, 0:512], ps[:, 0:512])
                        nc.vector.tensor_copy(t[:, 512:], ps[:, 512:])
                    else:
                        nc.scalar.copy(t[:], ps[:])
                    tiles[(side, p)] = t

                def emit_products():
                    for u, (a, b) in enumerate(PAIRS):
                        if (("Z", a) in tiles and ("W", b) in tiles
                                and u not in prod_state):
                            prod_state[u] = True
                            n = prod_state["n"]
                            prod_state["n"] = n + 1
                            P_t = prod.tile([128, 1024], BF16, tag="pair")
                            eng = nc.gpsimd if n < 2 else nc.vector
                            eng.tensor_tensor(
                                P_t[:], tiles[("Z", a)][:], tiles[("W", b)][:],
                                mybir.AluOpType.mult,
                            )
                            # i-fold into trace strip rows 8q+s, (c,j) cols
                            strip, q = u // 4, u % 4
                            qlast = 3 if strip < 2 else 1
                            for h in range(0, 16 * G, 512):
                                nc.tensor.matmul(
                                    ps_tr[32 * strip: 32 * strip + 32,
                                          h: h + 512],
                                    sel4_t[:, 32 * q: 32 * q + 32],
                                    P_t[:, h: h + 512],
                                    start=(q == 0),
                                    stop=(q == qlast),
                                    tile_position=(0, 32 * strip),
                                )

                emit_products()                              # (1,1)
                chain("Z", 2, xe_v, ("Z", 1), True)
                emit_products()                              # (2,1)
                chain("Z", 3, xe_v, ("Z", 2), True)
                emit_products()                              # (3,1)

                # expand z3 into its block-diag slab (on-chip scatter)
                z3_t = tiles[("Z", 3)]
                for s in range(8):
                    qs[s % 3].dma_start(
                        z3e_v[16 * s: 16 * s + 16, :, 16 * s: 16 * s + 16],
                        z3_t[16 * s: 16 * s + 16, :].rearrange(
                            "p (c j) -> p c j", j=16),
                    )

                chain("Z", 4, xe_v, ("Z", 3), True)
                chain("W", 4, z3e_v, ("W", 1), True)
                emit_products()                              # (1,4)(2,4)(3,4)
                chain("W", 7, z3e_v, ("W", 4), False)
                emit_products()                              # (1,7)..(4,7)

                # j-fold 16-col segments: [96, (c,j)] -> [96, c]
                nc.vector.tensor_reduce(
                    T_t[:, G * ch: G * (ch + 1)],
                    ps_tr.rearrange("p (c j) -> p c j", j=16),
                    mybir.AxisListType.X,
                    mybir.AluOpType.add,
                )

            # features: S = T/256, G_acc = sum_j W[:,j] * S^(j+1)
            S_t = fin.tile([96, CH * G], F32, tag="S")
            nc.vector.tensor_scalar_mul(S_t[:], T_t[:], 1.0 / 256.0)
            S2_t = fin.tile([96, CH * G], F32, tag="S2")
            nc.vector.tensor_tensor(S2_t[:], S_t[:], S_t[:], mybir.AluOpType.mult)
            S3_t = fin.tile([96, CH * G], F32, tag="S3")
            nc.gpsimd.tensor_tensor(S3_t[:], S2_t[:], S_t[:], mybir.AluOpType.mult)
            S4_t = fin.tile([96, CH * G], F32, tag="S4")
            nc.vector.tensor_tensor(S4_t[:], S2_t[:], S2_t[:], mybir.AluOpType.mult)

            G1_t = fin.tile([96, CH * G], F32, tag="G1")
            nc.vector.tensor_scalar(
                G1_t[:], S_t[:], wmat_t[:, 0:1], None, mybir.AluOpType.mult
            )
            G2_t = fin.tile([96, CH * G], F32, tag="G2")
            nc.vector.scalar_tensor_tensor(
                G2_t[:], S2_t[:], wmat_t[:, 1:2], G1_t[:],
                mybir.AluOpType.mult, mybir.AluOpType.add,
            )
            G3_t = fin.tile([96, CH * G], F32, tag="G3")
            nc.vector.scalar_tensor_tensor(
                G3_t[:], S3_t[:], wmat_t[:, 2:3], G2_t[:],
                mybir.AluOpType.mult, mybir.AluOpType.add,
            )
            G4_t = fin.tile([96, CH * G], F32, tag="G4")
            nc.vector.scalar_tensor_tensor(
                G4_t[:], S4_t[:], wmat_t[:, 3:4], G3_t[:],
                mybir.AluOpType.mult, mybir.AluOpType.add,
            )

            # fold the 96 rows into 8 sample rows: out[s, (ch,c)]
            ps_out = psum_c.tile([8, CH * G], F32, tag="psz", name="ps_out")
            for h in range(0, CH * G, 512):
                nc.tensor.matmul(
                    ps_out[:, h: h + 512],
                    self8_t[:],
                    G4_t[:, h: h + 512],
                    start=True,
                    stop=True,
                )
            out_sb = fin.tile([8, CH * G], F32, tag="outsb")
            nc.vector.tensor_copy(out_sb[:], ps_out[:])
            # out[b], b = 512*ch + 8*c + s; cols are (ch,c)
            nc.sync.dma_start(
                out_d.rearrange("(ch c s) -> s (ch c)", ch=CH, c=G, s=8),
                out_sb[:],
            )

    nc.compile()
    return nc


def _prep_inputs(x: np.ndarray, coef: np.ndarray):
    x = np.ascontiguousarray(x, dtype=np.float32).reshape(B, N, N)
    xb = x.astype(ml_dtypes.bfloat16)
    # [core, ch, c, s, i, j]
    x6 = xb.reshape(M, CH, G, 8, N, N)
    xr = np.ascontiguousarray(x6.transpose(0, 1, 3, 4, 2, 5)).reshape(
        M, CH * 128, 1024
    )
    # zr[(ch,s,k),(c,i)] = x[ch,c,s,i,k]
    zr = np.ascontiguousarray(x6.transpose(0, 1, 3, 5, 2, 4)).reshape(
        M, CH * 128, 1024
    )
    sel4, self8, wmat = _consts(np.asarray(coef, dtype=np.float32))
    return [
        {"x": xr[i], "z": zr[i], "sel4": sel4, "self8": self8, "wmat": wmat}
        for i in range(M)
    ]


def kernel(x: np.ndarray, coef: np.ndarray) -> np.ndarray:
    global _cached
    if _cached is None:
        _cached = build()
    in_maps = _prep_inputs(x, coef)
    res = run_bass_kernel_spmd(_cached, in_maps, core_ids=list(range(M)))
    out = np.concatenate(
        [np.asarray(res.results[i]["out"]).reshape(BS) for i in range(M)]
    )
    return out.astype(np.float32)


if __name__ == "__main__":
    rng = np.random.default_rng(0)
    x = (rng.standard_normal((B, N, N)) * 0.5).astype(np.float32)
    coef = (rng.standard_normal((ROWS, COLS)) * np.sqrt(0.5)).astype(np.float32)
    got = kernel(x, coef)
    print(got[:8])


# revision 14
# speedup vs baseline: 1.6918x; 1.2929x over previous
"""Distributed trace-polynomial Bass kernel for trn2 (8 NeuronCores), v5.

Problem: x [65536,16,16] f32, coef [10,4].
  t_u(b) = trace(x_b^(u+2)), u=0..9
  out[b] = sum_uj coef[u,j] * t_u^(j+1) / 256^(u+j+1)

Math (per core, 8192 samples, bf16 tiles): two chains meet in the
middle, 5 matmuls per 8-sample group (baseline needed 9):
  Z-chain (transposed powers, Z_a[(s,i),(c,j)] = x^a[j,i]):
      z2 = S[x].z1,  z3 = S[x].z2,  z4 = S[x].z3
  W-chain (natural powers, W_b[(s,i),(c,j)] = x^b[i,j]):
      W4 = S[z3].x,  W7 = S[z3].W4
S[t] is an 8-sample block-diagonal stationary slab built from tile t.
Two slab types: S[x] (from DRAM) and S[z3] (scattered on-chip from
z3). Traces: tr(x^(a+b))[b] = <Z_a, W_b>_F per sample, pairs
  k=2..11: (1,1)(2,1)(3,1)(1,4)(2,4)(3,4)(1,7)(2,7)(3,7)(4,7).

Pipelining: the W-phase of chunk ch runs one loop iteration later than
its Z-phase, so the on-chip z3 slab scatter has a full chunk-period to
land; ingress DMAs prefetch one chunk ahead; slabs are triple
buffered. Products on DVE, i-fold on TE (sel4 selector matmuls into
[96,(c,j)] PSUM strips), one j-fold tensor_reduce per chunk, feature
endgame on DVE split in two halves (first half overlaps the tail
chunks).

Host ships x and z=x^T pre-reordered to partition-major
[ch][s][i][c][j] so compact loads are single contiguous DMAs.
Sharding: pure data parallel, batch split 8 ways across cores.
"""

import numpy as np
from contextlib import ExitStack

import ml_dtypes

from concourse import bass, bacc, tile, mybir
from concourse.bass_utils import run_bass_kernel_spmd

B, N = 65536, 16
ROWS, COLS = 10, 4
M = 8           # cores
BS = B // M     # 8192 samples per core
CH = 16         # chunks per core
CS = BS // CH   # 512 samples per chunk
G = CS // 8     # 64 groups (of 8 samples) per chunk

BF16 = mybir.dt.bfloat16
F32 = mybir.dt.float32

# trace pairs: k = a + b, Z-side a in {1,2,3,4}, W-side b in {1,4,7};
# pair u = k-2; Z-phase emits u=0..2, W-phase u=3..9
PAIRS = [(1, 1), (2, 1), (3, 1), (1, 4), (2, 4), (3, 4),
         (1, 7), (2, 7), (3, 7), (4, 7)]

_cached = None


def _consts(coef: np.ndarray):
    # row layout of the 96-partition trace tile: r = 32*(u//4) + 8*(u%4) + s
    wmat = np.zeros((96, COLS), np.float32)
    self8 = np.zeros((96, 8), np.float32)
    for u in range(ROWS):
        base = 32 * (u // 4) + 8 * (u % 4)
        for s in range(8):
            wmat[base + s, :] = coef[u, :] * (256.0 ** (-u))
            self8[base + s, s] = 1.0
    # sel4[:, 32q:32q+32]: lhsT mapping partition (s,i) -> within-slice col 8q+s
    sel4 = np.zeros((128, 128), np.float32)
    for q in range(4):
        for s in range(8):
            for i in range(16):
                sel4[16 * s + i, 32 * q + 8 * q + s] = 1.0
    return sel4.astype(ml_dtypes.bfloat16), self8, wmat


def build():
    nc = bacc.Bacc("TRN2", target_bir_lowering=False, debug=False, num_devices=M)

    # xr row (ch, s, i) = 128*ch + 16*s + i; col (c, j) = 16*c + j
    # xr[(ch,s,i), (c,j)] = x_{512ch+8c+s}[i, j]
    # zr[(ch,s,k), (c,i)] = x_{512ch+8c+s}[i, k]
    x_d = nc.dram_tensor("x", [CH * 128, 1024], BF16, kind="ExternalInput").ap()
    z_d = nc.dram_tensor("z", [CH * 128, 1024], BF16, kind="ExternalInput").ap()
    sel4_d = nc.dram_tensor("sel4", [128, 128], BF16, kind="ExternalInput").ap()
    self8_d = nc.dram_tensor("self8", [96, 8], F32, kind="ExternalInput").ap()
    wmat_d = nc.dram_tensor("wmat", [96, COLS], F32, kind="ExternalInput").ap()
    out_d = nc.dram_tensor("out", [BS], F32, kind="ExternalOutput").ap()

    NB = 3  # slab buffers

    with tile.TileContext(nc) as tc:
        with ExitStack() as ctx:
            consts = ctx.enter_context(tc.tile_pool(name="consts", bufs=1))
            xexp_p = ctx.enter_context(tc.tile_pool(name="xexp", bufs=1))
            data = ctx.enter_context(tc.tile_pool(name="data", bufs=3))
            pows = ctx.enter_context(tc.tile_pool(name="pows", bufs=2))
            prod = ctx.enter_context(tc.tile_pool(name="prod", bufs=4))
            psum_c = ctx.enter_context(tc.tile_pool(name="psumc", bufs=2, space="PSUM"))
            psum_t = ctx.enter_context(tc.tile_pool(name="psumt", bufs=2, space="PSUM"))
            trc = ctx.enter_context(tc.tile_pool(name="trace", bufs=1))
            fin = ctx.enter_context(tc.tile_pool(name="fin", bufs=1))

            sel4_t = consts.tile([128, 128], BF16)
            nc.sync.dma_start(sel4_t[:], sel4_d[:])
            self8_t = consts.tile([96, 8], F32)
            nc.sync.dma_start(self8_t[:], self8_d[:])
            wmat_t = consts.tile([96, COLS], F32)
            nc.sync.dma_start(wmat_t[:], wmat_d[:])

            # trace accumulator [96, CH*G] f32
            T_t = trc.tile([96, CH * G], F32)

            # block-diag stationary slabs ([c][s][j] cols; BIR wants a single
            # free dim on the stationary, so group slabs are contiguous
            # 128-col blocks); off-block zeros are written once and persist.
            xe_ts = [xexp_p.tile([128, G * 128], BF16, tag=f"xe{i}", name=f"xe{i}")
                     for i in range(NB)]
            z3e_ts = [xexp_p.tile([128, G * 128], BF16, tag=f"z3e{i}",
                                  name=f"z3e{i}") for i in range(NB)]
            for i, t in enumerate(xe_ts + z3e_ts):
                [nc.gpsimd, nc.vector][i % 2].memset(t[:], 0.0)

            def exp_view(t):
                return t.rearrange("p (c w) -> p c w", w=128)

            st = {}  # per-chunk state: tiles dict + ps_tr

            def ingress(ch):
                xrows = x_d[128 * ch: 128 * (ch + 1), :]
                zrows = z_d[128 * ch: 128 * (ch + 1), :]
                xc_t = data.tile([128, 1024], BF16, tag="xc")
                nc.sync.dma_start(xc_t[:], xrows)
                zc_t = data.tile([128, 1024], BF16, tag="zc")
                nc.gpsimd.dma_start(zc_t[:], zrows)
                xe_v = exp_view(xe_ts[ch % NB])
                for s in range(8):
                    # 6 on sync, 2 on gpsimd
                    q = nc.sync if s < 6 else nc.gpsimd
                    q.dma_start(
                        xe_v[16 * s: 16 * s + 16, :, 16 * s: 16 * s + 16],
                        xrows[16 * s: 16 * s + 16, :].rearrange(
                            "p (c j) -> p c j", j=16),
                    )
                st[ch] = {"tiles": {("Z", 1): zc_t, ("W", 1): xc_t}}

            def chain(ch, side, p, stat_v, rhs_key, act_only=False):
                tiles = st[ch]["tiles"]
                ps = psum_c.tile([128, 1024], F32, tag="psz")
                rhs = tiles[rhs_key]
                for c in range(G):
                    nc.tensor.matmul(
                        ps[:, 16 * c: 16 * c + 16],
                        stat_v[:, c],
                        rhs[:, 16 * c: 16 * c + 16],
                        start=True,
                        stop=True,
                    )
                t = pows.tile([128, 1024], BF16, tag=f"{side}{p}")
                if act_only:
                    nc.scalar.copy(t[:], ps[:])
                else:
                    nc.scalar.copy(t[:, 0:512], ps[:, 0:512])
                    nc.vector.tensor_copy(t[:, 512:], ps[:, 512:])
                tiles[(side, p)] = t

            def fold(ch, u):
                tiles = st[ch]["tiles"]
                a, b = PAIRS[u]
                P_t = prod.tile([128, 1024], BF16, tag="pair")
                nc.vector.tensor_tensor(
                    P_t[:], tiles[("Z", a)][:], tiles[("W", b)][:],
                    mybir.AluOpType.mult,
                )
                strip, q = u // 4, u % 4
                qlast = 3 if strip < 2 else 1
                ps_tr = st[ch]["ps_tr"]
                for h in range(0, 16 * G, 512):
                    nc.tensor.matmul(
                        ps_tr[32 * strip: 32 * strip + 32, h: h + 512],
                        sel4_t[:, 32 * q: 32 * q + 32],
                        P_t[:, h: h + 512],
                        start=(q == 0),
                        stop=(q == qlast),
                        tile_position=(0, 32 * strip),
                    )

            def z_phase(ch):
                xe_v = exp_view(xe_ts[ch % NB])
                z3e_v = exp_view(z3e_ts[ch % NB])
                st[ch]["ps_tr"] = psum_t.tile([96, 16 * G], F32, tag="pstr", name=f"pstr{ch}")
                fold(ch, 0)                       # (1,1)
                chain(ch, "Z", 2, xe_v, ("Z", 1))
                fold(ch, 1)                       # (2,1)
                chain(ch, "Z", 3, xe_v, ("Z", 2))
                fold(ch, 2)                       # (3,1)
                # scatter z3 into its block-diag slab (consumed next iter)
                z3_t = st[ch]["tiles"][("Z", 3)]
                for s in range(8):
                    q = nc.gpsimd if s < 4 else nc.scalar
                    q.dma_start(
                        z3e_v[16 * s: 16 * s + 16, :, 16 * s: 16 * s + 16],
                        z3_t[16 * s: 16 * s + 16, :].rearrange(
                            "p (c j) -> p c j", j=16),
                    )
                chain(ch, "Z", 4, xe_v, ("Z", 3))

            def w_phase(ch):
                z3e_v = exp_view(z3e_ts[ch % NB])
                chain(ch, "W", 4, z3e_v, ("W", 1))
                fold(ch, 3)                       # (1,4)
                fold(ch, 4)                       # (2,4)
                fold(ch, 5)                       # (3,4)
                chain(ch, "W", 7, z3e_v, ("W", 4), act_only=True)
                for u in range(6, 10):            # (1,7)(2,7)(3,7)(4,7)
                    fold(ch, u)
                # j-fold 16-col segments: [96, (c,j)] -> [96, c]
                nc.vector.tensor_reduce(
                    T_t[:, G * ch: G * (ch + 1)],
                    st[ch]["ps_tr"].rearrange("p (c j) -> p c j", j=16),
                    mybir.AxisListType.X,
                    mybir.AluOpType.add,
                )
                del st[ch]

            def endgame(lo, hi, G4_t):
                # features on T_t[:, lo:hi]: S = T/256, G = sum_j w_j S^(j+1)
                w = hi - lo
                S_t = fin.tile([96, w], F32, tag="S")
                nc.vector.tensor_scalar_mul(S_t[:], T_t[:, lo:hi], 1.0 / 256.0)
                S2_t = fin.tile([96, w], F32, tag="S2")
                nc.vector.tensor_tensor(S2_t[:], S_t[:], S_t[:],
                                        mybir.AluOpType.mult)
                S3_t = fin.tile([96, w], F32, tag="S3")
                nc.gpsimd.tensor_tensor(S3_t[:], S2_t[:], S_t[:],
                                        mybir.AluOpType.mult)
                S4_t = fin.tile([96, w], F32, tag="S4")
                nc.gpsimd.tensor_tensor(S4_t[:], S2_t[:], S2_t[:],
                                        mybir.AluOpType.mult)
                G1_t = fin.tile([96, w], F32, tag="G1")
                nc.vector.tensor_scalar(
                    G1_t[:], S_t[:], wmat_t[:, 0:1], None, mybir.AluOpType.mult
                )
                G2_t = fin.tile([96, w], F32, tag="G2")
                nc.vector.scalar_tensor_tensor(
                    G2_t[:], S2_t[:], wmat_t[:, 1:2], G1_t[:],
                    mybir.AluOpType.mult, mybir.AluOpType.add,
                )
                G3_t = fin.tile([96, w], F32, tag="G3")
                nc.vector.scalar_tensor_tensor(
                    G3_t[:], S3_t[:], wmat_t[:, 2:3], G2_t[:],
                    mybir.AluOpType.mult, mybir.AluOpType.add,
                )
                nc.vector.scalar_tensor_tensor(
                    G4_t[:], S4_t[:], wmat_t[:, 3:4], G3_t[:],
                    mybir.AluOpType.mult, mybir.AluOpType.add,
                )

            G4a_t = fin.tile([96, 512], F32, tag="G4a")
            G4b_t = fin.tile([96, 512], F32, tag="G4b")
            ingress(0)
            for ch in range(CH + 1):
                if ch < CH:
                    if ch + 1 < CH:
                        ingress(ch + 1)
                    z_phase(ch)
                if ch >= 1:
                    w_phase(ch - 1)
                if ch == 9:
                    # chunks 0-7 traces are final; overlap half the features
                    endgame(0, 512, G4a_t)
            endgame(512, 1024, G4b_t)

            # fold the 96 rows into 8 sample rows: out[s, (ch,c)]
            ps_out = psum_c.tile([8, CH * G], F32, tag="psz", name="ps_out")
            for lo, g4 in ((0, G4a_t), (512, G4b_t)):
                for h in range(0, 512, 512):
                    nc.tensor.matmul(
                        ps_out[:, lo + h: lo + h + 512],
                        self8_t[:],
                        g4[:, h: h + 512],
                        start=True,
                        stop=True,
                    )
            out_sb = fin.tile([8, CH * G], F32, tag="outsb")
            nc.vector.tensor_copy(out_sb[:], ps_out[:])
            # out[b], b = 512*ch + 8*c + s; cols are (ch,c)
            nc.sync.dma_start(
                out_d.rearrange("(ch c s) -> s (ch c)", ch=CH, c=G, s=8),
                out_sb[:],
            )

    nc.compile()
    return nc


def _prep_inputs(x: np.ndarray, coef: np.ndarray):
    x = np.ascontiguousarray(x, dtype=np.float32).reshape(B, N, N)
    xb = x.astype(ml_dtypes.bfloat16)
    # [core, ch, c, s, i, j]
    x6 = xb.reshape(M, CH, G, 8, N, N)
    xr = np.ascontiguousarray(x6.transpose(0, 1, 3, 4, 2, 5)).reshape(
        M, CH * 128, 1024
    )
    # zr[(ch,s,k),(c,i)] = x[ch,c,s,i,k]
    zr = np.ascontiguousarray(x6.transpose(0, 1, 3, 5, 2, 4)).reshape(
        M, CH * 128, 1024
    )
    sel4, self8, wmat = _consts(np.asarray(coef, dtype=np.float32))
    return [
        {"x": xr[i], "z": zr[i], "sel4": sel4, "self8": self8, "wmat": wmat}
        for i in range(M)
    ]


def kernel(x: np.ndarray, coef: np.ndarray) -> np.ndarray:
    global _cached
    if _cached is None:
        _cached = build()
    in_maps = _prep_inputs(x, coef)
    res = run_bass_kernel_spmd(_cached, in_maps, core_ids=list(range(M)))
    out = np.concatenate(
        [np.asarray(res.results[i]["out"]).reshape(BS) for i in range(M)]
    )
    return out.astype(np.float32)


if __name__ == "__main__":
    rng = np.random.default_rng(0)
    x = (rng.standard_normal((B, N, N)) * 0.5).astype(np.float32)
    coef = (rng.standard_normal((ROWS, COLS)) * np.sqrt(0.5)).astype(np.float32)
    got = kernel(x, coef)
    print(got[:8])
